# revision 14
# baseline (speedup 1.0000x reference)
"""Multi-head attention (B=2, S=2048, D=2048, H=16, hd=128) on 8 NeuronCores.

Sharding: core c -> (batch b = c // 4, head group g = c % 4), 4 heads per
group.  Each core computes the QKV projection for its heads, causal
attention, and a partial output projection (W_out row-sharded by head).
The host sums the 4 partials per batch (the "all-reduce") and adds b_out.

Device layout (per core):
  inputs (host-prepared):
    xT    [D, S]      x[b].T               (f32r)
    wqk   [D, 1024]   [q(h0),k(h0),...,q(h3),k(h3)] col-blocks of W_qkv (f32r)
    wv    [D, 512]    [v(h0)..v(h3)] col-blocks of W_qkv               (f32r)
    wout  [512, D]    W_out rows for the 4 heads                       (f32r)
    bqk   [128, 8]    b_qkv per qk f-tile (bias along partitions)      (f32)
    bv    [128, 512]  b_qkv v-slice broadcast down partitions          (f32)
    maskm [128, 896]  causal staircase master mask                     (f32r)
    onescol [128, 1], ones1 [1, 128]                                   (f32r)
  output:
    out   [S, D]      partial (pre-all-reduce) output projection       (f32)

Phases per 512-token chunk c (emission order; Tile schedules/overlaps):
  1. QK^T: psum[feat 128, tok 512] = sum_k wqk_tile.T @ xT_tile -> q/k SBUF
     V:    psum[tok 128, vfeat 512] = sum_k xT_tile.T @ wv      -> v SBUF
  2. attention per head h: for sk tile j: scoresT = k_blk.T @ q_chunk;
     pT = exp(scoresT * scale) (ACT); diag tiles masked (DVE);
     rowsum += ones.T @ pT (M=1); avT += v_blk.T @ pT.
     rec = 1/rowsum (DVE); bcast = ones1.T @ rec (K=1 matmul);
     attnT = avT * bcast (ACT copy + DVE mul).
  3. out-proj: psum[tok 128, dmodel 512] = sum_h attnT_blk.T @ wout_blk
     -> DVE copy -> DMA to out.
"""

import numpy as np
import ml_dtypes

import concourse.bass as bass
import concourse.mybir as mybir
import concourse.tile as tile
from concourse.vector_clock import ScopedClock

DIM = 2048
HEADS = 16
HD = 128
B, S = 2, 2048
HPG = 4          # heads per group (per core)
CH = 512         # token chunk
NCH = S // CH    # 4
KT = DIM // 128  # 16 contraction tiles
SCALE = 1.0 / np.sqrt(HD)

F32 = mybir.dt.float32
F32R = mybir.dt.float32r
BF16 = mybir.dt.bfloat16

_PATCHED = False


def _patch_tile_for_neuronxcc():
    """This neuronxcc walrus accepts at most one sync-wait per instruction.
    Split multi-wait instructions (and the kernel-tail drain) into
    single-wait EventSemaphore/Drain instructions."""
    global _PATCHED
    if _PATCHED:
        return
    _PATCHED = True

    orig_add = tile.TileContext._add_instruction

    def _add_instruction(self, inst):
        si = getattr(inst, "sync_info", None)
        waits = list(si.on_wait) if si and si.on_wait else []
        if len(waits) > 1:
            for w in waits[:-1]:
                ev = mybir.InstEventSemaphore(
                    name=f"I-{self.nc.next_id()}-waitsplit", ins=[], outs=[]
                )
                ev.engine = inst.engine
                ev.sync_info = mybir.SyncInfo(on_wait=[w], on_update=[])
                orig_add(self, ev)
            inst.sync_info.on_wait = waits[-1:]
        orig_add(self, inst)

    tile.TileContext._add_instruction = _add_instruction

    def _drain_and_barrier(self, tick_clock, wait_clock):
        drain_inst = self.nc.sync.drain()
        wait_clock.add_sem_waits(
            drain_inst.ins, ScopedClock({None: tick_clock.global_clock})
        )
        waits = list(drain_inst.ins.sync_info.on_wait or [])
        if len(waits) > 1:
            drain_inst.ins.sync_info.on_wait = waits[:1]
            for w in waits[1:]:
                d2 = self.nc.sync.drain()
                d2.ins.sync_info = mybir.SyncInfo(on_wait=[w], on_update=[])
        self.nc.all_engine_barrier()
        popped = self.nc._tile_sem_poison_stack.pop()
        assert popped is self._sem_poison
        self.nc.clear_and_free_semaphores(list(self.sems.allocated().values()))
        self.nc.all_engine_barrier()

    tile.TileContext._drain_and_barrier = _drain_and_barrier


def build_nc():
    _patch_tile_for_neuronxcc()
    nc = bass.Bass()
    xT_d = nc.dram_tensor("xT", [DIM, S], BF16, kind="ExternalInput")
    wqk_d = nc.dram_tensor("wqk", [DIM, 2 * HPG * 128], BF16, kind="ExternalInput")
    wv_d = nc.dram_tensor("wv", [DIM, HPG * 128], BF16, kind="ExternalInput")
    wout_d = nc.dram_tensor("wout", [HPG * 128, DIM], BF16, kind="ExternalInput")
    bqk_d = nc.dram_tensor("bqk", [128, 2 * HPG], F32, kind="ExternalInput")
    bv_d = nc.dram_tensor("bv", [128, HPG * 128], F32, kind="ExternalInput")
    maskm_d = nc.dram_tensor("maskm", [128, 896], BF16, kind="ExternalInput")
    onesf_d = nc.dram_tensor("onesf", [128, 128], BF16, kind="ExternalInput")
    out_d = nc.dram_tensor("out", [S, DIM], F32, kind="ExternalOutput")

    with tile.TileContext(nc) as tc:
        with (
            tc.tile_pool(name="const", bufs=1) as const,
            tc.tile_pool(name="resid", bufs=1) as resid,
            tc.tile_pool(name="xt", bufs=6) as xtp,
            tc.tile_pool(name="wqk", bufs=3) as wqkp,
            tc.tile_pool(name="qch", bufs=2) as qchp,
            tc.tile_pool(name="pT", bufs=KT + 4) as pTp,
            tc.tile_pool(name="attn", bufs=2) as attnp,
            tc.tile_pool(name="stage", bufs=4) as stagep,
            tc.tile_pool(name="mm", bufs=4, space="PSUM") as mmp,
            tc.tile_pool(name="av", bufs=2, space="PSUM") as avp,
            tc.tile_pool(name="dps", bufs=2, space="PSUM") as dpsp,
        ):
            # constants
            bqk = const.tile([128, 2 * HPG], F32)
            bv = const.tile([128, HPG * 128], F32)
            maskm = const.tile([128, 896], BF16)
            onesf = const.tile([128, 128], BF16)
            nc.sync.dma_start(out=bqk, in_=bqk_d[:, :])
            nc.sync.dma_start(out=bv, in_=bv_d[:, :])
            nc.sync.dma_start(out=maskm, in_=maskm_d[:, :])
            nc.sync.dma_start(out=onesf, in_=onesf_d[:, :])

            # resident tensors
            k_sb = resid.tile([128, HPG, S], BF16)       # kT per head
            v_sb = resid.tile([128, KT, HPG * 128], BF16)  # v  [skp, sktile, vfeat]
            wv_sb = resid.tile([128, KT, HPG * 128], BF16)
            nc.sync.dma_start(
                out=wv_sb, in_=wv_d.ap().rearrange("(k p) f -> p k f", p=128)
            )

            wout_sb = resid.tile([128, HPG, DIM], BF16)
            nc.sync.dma_start(
                out=wout_sb, in_=wout_d.ap().rearrange("(k p) d -> p k d", p=128)
            )
            wqk_r = wqk_d.ap().rearrange("(k p) f -> p k f", p=128)

            q_chs, attn_chs = {}, {}

            xT_r = xT_d.ap().rearrange("(k p) t -> p k t", p=128)

            def phase1(c):
                # 4 batched x loads (4 k-tiles per DMA descriptor)
                xt4s = []
                for kk in range(4):
                    xt4 = xtp.tile([128, 4, CH], BF16, name="xt")
                    nc.sync.dma_start(
                        out=xt4,
                        in_=xT_r[:, 4 * kk:4 * kk + 4, c * CH:(c + 1) * CH],
                    )
                    xt4s.append(xt4)
                xts = [xt4s[k // 4][:, k % 4, :] for k in range(KT)]

                q_ch = qchp.tile([128, HPG, CH], BF16, name="qch")
                q_chs[c] = q_ch
                for ft in range(2 * HPG):
                    w_t = wqkp.tile([128, KT, 128], BF16, name="wqk")
                    nc.sync.dma_start(
                        out=w_t, in_=wqk_r[:, :, ft * 128:(ft + 1) * 128]
                    )
                    ps = mmp.tile([128, CH], F32, name="mm")
                    for k in range(KT):
                        nc.tensor.matmul(
                            ps[:, :], w_t[:, k, :], xts[k][:, :],
                            start=(k == 0), stop=(k == KT - 1),
                        )
                    hl, is_k = ft // 2, ft % 2
                    dst = (
                        k_sb[:, hl, c * CH:(c + 1) * CH] if is_k
                        else q_ch[:, hl, :]
                    )
                    nc.scalar.activation(
                        dst, ps[:, :], mybir.ActivationFunctionType.Identity,
                        bias=bqk[:, ft:ft + 1],
                    )

                for tt in range(4):
                    ps = mmp.tile([128, HPG * 128], F32, name="mm")
                    for k in range(KT):
                        nc.tensor.matmul(
                            ps[:, :], xts[k][:, tt * 128:(tt + 1) * 128],
                            wv_sb[:, k, :],
                            start=(k == 0), stop=(k == KT - 1),
                        )
                    nc.vector.tensor_add(v_sb[:, 4 * c + tt, :], ps[:, :], bv)

            def attention(c):
                # Head h's matmul block runs before head h-1's softmax tail
                # (broadcast matmul) so the PE never waits on ACT/DVE.
                q_ch = q_chs[c]
                attn_ch = attnp.tile([128, HPG, CH], BF16, name="attn")
                attn_chs[c] = attn_ch
                nsk = 4 * (c + 1)

                def _scores(h, j):
                    ps_s = mmp.tile([128, CH], F32, name="mm")
                    nc.tensor.matmul(
                        ps_s[:, :], k_sb[:, h, j * 128:(j + 1) * 128],
                        q_ch[:, h, :], start=True, stop=True,
                    )
                    return ps_s

                for h in range(HPG):
                    ps_av = avp.tile([128, CH], F32, name="av")
                    ps_d = dpsp.tile([128, CH], F32, name="dps")
                    pTs = []
                    ps_s_next = _scores(h, 0)
                    for j in range(nsk):
                        ps_s = ps_s_next
                        pT = pTp.tile([128, CH], BF16, name="pT")
                        nc.scalar.activation(
                            pT, ps_s[:, :], mybir.ActivationFunctionType.Exp,
                            scale=float(SCALE),
                        )
                        if j + 1 < nsk:
                            ps_s_next = _scores(h, j + 1)
                        jj = j - 4 * c
                        if jj >= 0:
                            off = 384 - 128 * jj
                            nc.vector.tensor_mul(
                                pT, pT, maskm[:, off:off + CH]
                            )
                        nc.tensor.matmul(
                            ps_av[:, :], v_sb[:, j, h * 128:(h + 1) * 128], pT,
                            start=(j == 0), stop=(j == nsk - 1),
                        )
                        pTs.append(pT)
                    # rowsum sweep: same ones weights for all j, and the
                    # result lands already replicated across partitions
                    for j in range(nsk):
                        nc.tensor.matmul(
                            ps_d[:, :], onesf, pTs[j],
                            start=(j == 0), stop=(j == nsk - 1),
                        )
                    rec_bc = stagep.tile([128, CH], F32, name="stage")
                    nc.vector.reciprocal(rec_bc, ps_d[:, :])
                    nc.vector.tensor_mul(attn_ch[:, h, :], ps_av[:, :],
                                         rec_bc)

            def outproj(c):
                attn_ch = attn_chs[c]
                for d in range(4):
                    for tt in range(4):
                        ps = mmp.tile([128, CH], F32, name="mm")
                        for kf in range(HPG):
                            nc.tensor.matmul(
                                ps[:, :],
                                attn_ch[:, kf, tt * 128:(tt + 1) * 128],
                                wout_sb[:, kf, d * CH:(d + 1) * CH],
                                start=(kf == 0), stop=(kf == HPG - 1),
                            )
                        st = stagep.tile([128, CH], F32, name="stage")
                        nc.vector.tensor_copy(st, ps[:, :])
                        nc.sync.dma_start(
                            out=out_d[c * CH + tt * 128:c * CH + (tt + 1) * 128,
                                      d * CH:(d + 1) * CH],
                            in_=st,
                        )

            # Emission order: phase1(c+1) sits between attention(c)'s tail
            # and outproj(c), keeping the PE dense across phase boundaries.
            phase1(0)
            attention(0)
            for c in range(1, NCH):
                phase1(c)
                outproj(c - 1)
                attention(c)
            outproj(NCH - 1)
    return nc


def make_in_maps(x, W_qkv, b_qkv, W_out):
    """Host-side sharding: per-core input dict."""
    x = np.asarray(x, dtype=np.float32)
    W_qkv = np.asarray(W_qkv, dtype=np.float32)
    b_qkv = np.asarray(b_qkv, dtype=np.float32)
    W_out = np.asarray(W_out, dtype=np.float32)

    maskm = np.zeros((128, 896), np.float32)
    pp, cc = np.mgrid[0:128, 0:896]
    maskm[(cc - pp) >= 384] = 1.0

    in_maps = []
    for core in range(8):
        b, g = core // 4, core % 4
        heads = [4 * g + i for i in range(HPG)]
        xT = np.ascontiguousarray(x[b].T)  # [D, S]

        qk_cols, v_cols = [], []
        for h in heads:
            base = h * 3 * HD
            qk_cols.extend(range(base, base + HD))          # q
            qk_cols.extend(range(base + HD, base + 2 * HD))  # k
            v_cols.extend(range(base + 2 * HD, base + 3 * HD))
        qk_cols = np.array(qk_cols)
        v_cols = np.array(v_cols)

        wqk = np.ascontiguousarray(W_qkv[:, qk_cols])   # [D, 1024]
        wv = np.ascontiguousarray(W_qkv[:, v_cols])     # [D, 512]
        rows = np.concatenate([np.arange(h * HD, (h + 1) * HD) for h in heads])
        wout = np.ascontiguousarray(W_out[rows, :])     # [512, D]

        bqk = np.ascontiguousarray(
            b_qkv[qk_cols].reshape(2 * HPG, 128).T)     # [128, 8]
        bv = np.tile(b_qkv[v_cols][None, :], (128, 1))  # [128, 512]

        bf = ml_dtypes.bfloat16
        in_maps.append({
            "xT": xT.astype(bf), "wqk": wqk.astype(bf), "wv": wv.astype(bf),
            "wout": wout.astype(bf),
            "bqk": bqk, "bv": bv, "maskm": maskm.astype(bf),
            "onesf": np.ones((128, 128), ml_dtypes.bfloat16),
        })
    return in_maps


def reduce_outputs(results, b_out):
    """Sum the 4 per-head-group partials per batch; add b_out."""
    b_out = np.asarray(b_out, dtype=np.float32)
    out = np.zeros((B, S, DIM), np.float32)
    for core in range(8):
        out[core // 4] += results[core]["out"]
    out += b_out[None, None, :]
    return out


def host_simulate(x, W_qkv, b_qkv, W_out, b_out):
    """Numpy mirror of the exact device decomposition (for layout checks)."""
    in_maps = make_in_maps(x, W_qkv, b_qkv, W_out)
    results = []
    for core in range(8):
        m = in_maps[core]
        xT, wqk, wv, wout = m["xT"], m["wqk"], m["wv"], m["wout"]
        bqk, bv, maskm = m["bqk"], m["bv"], m["maskm"]
        qkT = wqk.T @ xT + bqk.T.reshape(-1, 1)  # [1024, S]
        v = xT.T @ wv + bv[0][None, :]           # [S, 512]
        out = np.zeros((S, DIM), np.float32)
        attnT = np.zeros((512, S), np.float32)
        for h in range(HPG):
            qT = qkT[2 * h * 128:(2 * h + 1) * 128]      # [128, S]
            kT = qkT[(2 * h + 1) * 128:(2 * h + 2) * 128]
            for c in range(NCH):
                nsk = 4 * (c + 1)
                q_c = qT[:, c * CH:(c + 1) * CH]
                ps_av = np.zeros((128, CH), np.float32)
                ps_d = np.zeros((1, CH), np.float32)
                for j in range(nsk):
                    sT = kT[:, j * 128:(j + 1) * 128].T @ q_c
                    pT = np.exp(sT * SCALE)
                    jj = j - 4 * c
                    if jj >= 0:
                        off = 384 - 128 * jj
                        pT = pT * maskm[:, off:off + CH]
                    ps_d += pT.sum(axis=0, keepdims=True)
                    ps_av += v[j * 128:(j + 1) * 128,
                               h * 128:(h + 1) * 128].T @ pT
                attnT[h * 128:(h + 1) * 128, c * CH:(c + 1) * CH] = (
                    ps_av / ps_d)
        out = attnT.T @ wout  # [S, D]
        results.append({"out": out})
    return reduce_outputs(results, b_out)


_NC = None


def kernel(x, mask, W_qkv, b_qkv, W_out, b_out):
    global _NC
    from concourse.bass_utils import run_bass_kernel_spmd
    if _NC is None:
        _NC = build_nc()
    in_maps = make_in_maps(x, W_qkv, b_qkv, W_out)
    res = run_bass_kernel_spmd(_NC, in_maps, core_ids=list(range(8)))
    return reduce_outputs(res.results, b_out)


# revision 15
# speedup vs baseline: 1.1645x; 1.1645x over previous
"""Multi-head attention (B=2, S=2048, D=2048, H=16, hd=128) on 8 NeuronCores.

Sharding: core c -> (batch b = c // 4, head group g = c % 4), 4 heads per
group.  Each core computes the QKV projection for its heads, causal
attention, and a partial output projection (W_out row-sharded by head).
The host sums the 4 partials per batch (the "all-reduce") and adds b_out.

Device layout (per core):
  inputs (host-prepared):
    xT    [D, S]      x[b].T               (f32r)
    wqk   [D, 1024]   [q(h0),k(h0),...,q(h3),k(h3)] col-blocks of W_qkv (f32r)
    wv    [D, 512]    [v(h0)..v(h3)] col-blocks of W_qkv               (f32r)
    wout  [512, D]    W_out rows for the 4 heads                       (f32r)
    bqk   [128, 8]    b_qkv per qk f-tile (bias along partitions)      (f32)
    bv    [128, 512]  b_qkv v-slice broadcast down partitions          (f32)
    maskm [128, 896]  causal staircase master mask                     (f32r)
    onescol [128, 1], ones1 [1, 128]                                   (f32r)
  output:
    out   [S, D]      partial (pre-all-reduce) output projection       (f32)

Phases per 512-token chunk c (emission order; Tile schedules/overlaps):
  1. QK^T: psum[feat 128, tok 512] = sum_k wqk_tile.T @ xT_tile -> q/k SBUF
     V:    psum[tok 128, vfeat 512] = sum_k xT_tile.T @ wv      -> v SBUF
  2. attention per head h: for sk tile j: scoresT = k_blk.T @ q_chunk;
     pT = exp(scoresT * scale) (ACT); diag tiles masked (DVE);
     rowsum += ones.T @ pT (M=1); avT += v_blk.T @ pT.
     rec = 1/rowsum (DVE); bcast = ones1.T @ rec (K=1 matmul);
     attnT = avT * bcast (ACT copy + DVE mul).
  3. out-proj: psum[tok 128, dmodel 512] = sum_h attnT_blk.T @ wout_blk
     -> DVE copy -> DMA to out.
"""

import numpy as np
import ml_dtypes

import concourse.bass as bass
import concourse.mybir as mybir
import concourse.tile as tile
from concourse.vector_clock import ScopedClock

DIM = 2048
HEADS = 16
HD = 128
B, S = 2, 2048
HPG = 4          # heads per group (per core)
CH = 512         # token chunk
NCH = S // CH    # 4
KT = DIM // 128  # 16 contraction tiles
SCALE = 1.0 / np.sqrt(HD)

F32 = mybir.dt.float32
F32R = mybir.dt.float32r
BF16 = mybir.dt.bfloat16

_PATCHED = False


def _patch_tile_for_neuronxcc():
    """This neuronxcc walrus accepts at most one sync-wait per instruction.
    Split multi-wait instructions (and the kernel-tail drain) into
    single-wait EventSemaphore/Drain instructions."""
    global _PATCHED
    if _PATCHED:
        return
    _PATCHED = True

    orig_add = tile.TileContext._add_instruction

    def _add_instruction(self, inst):
        si = getattr(inst, "sync_info", None)
        waits = list(si.on_wait) if si and si.on_wait else []
        if len(waits) > 1:
            for w in waits[:-1]:
                ev = mybir.InstEventSemaphore(
                    name=f"I-{self.nc.next_id()}-waitsplit", ins=[], outs=[]
                )
                ev.engine = inst.engine
                ev.sync_info = mybir.SyncInfo(on_wait=[w], on_update=[])
                orig_add(self, ev)
            inst.sync_info.on_wait = waits[-1:]
        orig_add(self, inst)

    tile.TileContext._add_instruction = _add_instruction

    def _drain_and_barrier(self, tick_clock, wait_clock):
        drain_inst = self.nc.sync.drain()
        wait_clock.add_sem_waits(
            drain_inst.ins, ScopedClock({None: tick_clock.global_clock})
        )
        waits = list(drain_inst.ins.sync_info.on_wait or [])
        if len(waits) > 1:
            drain_inst.ins.sync_info.on_wait = waits[:1]
            for w in waits[1:]:
                d2 = self.nc.sync.drain()
                d2.ins.sync_info = mybir.SyncInfo(on_wait=[w], on_update=[])
        self.nc.all_engine_barrier()
        popped = self.nc._tile_sem_poison_stack.pop()
        assert popped is self._sem_poison
        self.nc.clear_and_free_semaphores(list(self.sems.allocated().values()))
        self.nc.all_engine_barrier()

    tile.TileContext._drain_and_barrier = _drain_and_barrier


def build_nc():
    _patch_tile_for_neuronxcc()
    nc = bass.Bass()
    xT_d = nc.dram_tensor("xT", [DIM, S], BF16, kind="ExternalInput")
    wqk_d = nc.dram_tensor("wqk", [DIM, 2 * HPG * 128], BF16, kind="ExternalInput")
    wv_d = nc.dram_tensor("wv", [DIM, HPG * 128], BF16, kind="ExternalInput")
    wout_d = nc.dram_tensor("wout", [HPG * 128, DIM], BF16, kind="ExternalInput")
    bqk_d = nc.dram_tensor("bqk", [128, 2 * HPG], F32, kind="ExternalInput")
    bv_d = nc.dram_tensor("bv", [128, HPG * 128], F32, kind="ExternalInput")
    maskm_d = nc.dram_tensor("maskm", [128, 896], BF16, kind="ExternalInput")
    onesf_d = nc.dram_tensor("onesf", [128, 128], BF16, kind="ExternalInput")
    out_d = nc.dram_tensor("out", [S, DIM], F32, kind="ExternalOutput")

    with tile.TileContext(nc) as tc:
        with (
            tc.tile_pool(name="const", bufs=1) as const,
            tc.tile_pool(name="resid", bufs=1) as resid,
            tc.tile_pool(name="xt", bufs=KT + 4) as xtp,
            tc.tile_pool(name="wqk", bufs=3) as wqkp,
            tc.tile_pool(name="qch", bufs=2) as qchp,
            tc.tile_pool(name="pT", bufs=KT + 4) as pTp,
            tc.tile_pool(name="attn", bufs=2) as attnp,
            tc.tile_pool(name="stage", bufs=4) as stagep,
            tc.tile_pool(name="mm", bufs=4, space="PSUM") as mmp,
            tc.tile_pool(name="av", bufs=2, space="PSUM") as avp,
            tc.tile_pool(name="dps", bufs=2, space="PSUM") as dpsp,
        ):
            # constants
            bqk = const.tile([128, 2 * HPG], F32)
            bv = const.tile([128, HPG * 128], F32)
            maskm = const.tile([128, 896], BF16)
            onesf = const.tile([128, 128], BF16)
            nc.sync.dma_start(out=bqk, in_=bqk_d[:, :])
            nc.sync.dma_start(out=bv, in_=bv_d[:, :])
            nc.sync.dma_start(out=maskm, in_=maskm_d[:, :])
            nc.sync.dma_start(out=onesf, in_=onesf_d[:, :])

            # resident tensors
            k_sb = resid.tile([128, HPG, S], BF16)       # kT per head
            v_sb = resid.tile([128, KT, HPG * 128], BF16)  # v  [skp, sktile, vfeat]
            wv_sb = resid.tile([128, KT, HPG * 128], BF16)
            nc.sync.dma_start(
                out=wv_sb, in_=wv_d.ap().rearrange("(k p) f -> p k f", p=128)
            )

            wout_sb = resid.tile([128, HPG, DIM], BF16)
            nc.sync.dma_start(
                out=wout_sb, in_=wout_d.ap().rearrange("(k p) d -> p k d", p=128)
            )
            wqk_r = wqk_d.ap().rearrange("(k p) f -> p k f", p=128)

            q_chs, attn_chs = {}, {}

            def phase1(c):
                xts = []
                for k in range(KT):
                    xt = xtp.tile([128, CH], BF16, name="xt")
                    nc.sync.dma_start(
                        out=xt, in_=xT_d[k * 128:(k + 1) * 128, c * CH:(c + 1) * CH]
                    )
                    xts.append(xt)

                q_ch = qchp.tile([128, HPG, CH], BF16, name="qch")
                q_chs[c] = q_ch
                for ft in range(2 * HPG):
                    w_t = wqkp.tile([128, KT, 128], BF16, name="wqk")
                    nc.sync.dma_start(
                        out=w_t, in_=wqk_r[:, :, ft * 128:(ft + 1) * 128]
                    )
                    ps = mmp.tile([128, CH], F32, name="mm")
                    for k in range(KT):
                        nc.tensor.matmul(
                            ps[:, :], w_t[:, k, :], xts[k][:, :],
                            start=(k == 0), stop=(k == KT - 1),
                        )
                    hl, is_k = ft // 2, ft % 2
                    dst = (
                        k_sb[:, hl, c * CH:(c + 1) * CH] if is_k
                        else q_ch[:, hl, :]
                    )
                    nc.scalar.activation(
                        dst, ps[:, :], mybir.ActivationFunctionType.Identity,
                        bias=bqk[:, ft:ft + 1],
                    )

                for tt in range(4):
                    ps = mmp.tile([128, HPG * 128], F32, name="mm")
                    for k in range(KT):
                        nc.tensor.matmul(
                            ps[:, :], xts[k][:, tt * 128:(tt + 1) * 128],
                            wv_sb[:, k, :],
                            start=(k == 0), stop=(k == KT - 1),
                        )
                    nc.vector.tensor_add(v_sb[:, 4 * c + tt, :], ps[:, :], bv)

            def attention(c):
                # Head h's matmul block runs before head h-1's softmax tail
                # (broadcast matmul) so the PE never waits on ACT/DVE.
                q_ch = q_chs[c]
                attn_ch = attnp.tile([128, HPG, CH], BF16, name="attn")
                attn_chs[c] = attn_ch
                nsk = 4 * (c + 1)

                def _scores(h, j):
                    ps_s = mmp.tile([128, CH], F32, name="mm")
                    nc.tensor.matmul(
                        ps_s[:, :], k_sb[:, h, j * 128:(j + 1) * 128],
                        q_ch[:, h, :], start=True, stop=True,
                    )
                    return ps_s

                for h in range(HPG):
                    ps_av = avp.tile([128, CH], F32, name="av")
                    ps_d = dpsp.tile([128, CH], F32, name="dps")
                    pTs = []
                    ps_s_next = _scores(h, 0)
                    for j in range(nsk):
                        ps_s = ps_s_next
                        pT = pTp.tile([128, CH], BF16, name="pT")
                        nc.scalar.activation(
                            pT, ps_s[:, :], mybir.ActivationFunctionType.Exp,
                            scale=float(SCALE),
                        )
                        if j + 1 < nsk:
                            ps_s_next = _scores(h, j + 1)
                        jj = j - 4 * c
                        if jj >= 0:
                            off = 384 - 128 * jj
                            nc.vector.tensor_mul(
                                pT, pT, maskm[:, off:off + CH]
                            )
                        nc.tensor.matmul(
                            ps_av[:, :], v_sb[:, j, h * 128:(h + 1) * 128], pT,
                            start=(j == 0), stop=(j == nsk - 1),
                        )
                        pTs.append(pT)
                    # rowsum sweep: same ones weights for all j, and the
                    # result lands already replicated across partitions
                    for j in range(nsk):
                        nc.tensor.matmul(
                            ps_d[:, :], onesf, pTs[j],
                            start=(j == 0), stop=(j == nsk - 1),
                        )
                    rec_bc = stagep.tile([128, CH], F32, name="stage")
                    nc.vector.reciprocal(rec_bc, ps_d[:, :])
                    nc.vector.tensor_mul(attn_ch[:, h, :], ps_av[:, :],
                                         rec_bc)

            def outproj(c):
                attn_ch = attn_chs[c]
                for d in range(4):
                    for tt in range(4):
                        ps = mmp.tile([128, CH], F32, name="mm")
                        for kf in range(HPG):
                            nc.tensor.matmul(
                                ps[:, :],
                                attn_ch[:, kf, tt * 128:(tt + 1) * 128],
                                wout_sb[:, kf, d * CH:(d + 1) * CH],
                                start=(kf == 0), stop=(kf == HPG - 1),
                            )
                        st = stagep.tile([128, CH], F32, name="stage")
                        nc.vector.tensor_copy(st, ps[:, :])
                        nc.sync.dma_start(
                            out=out_d[c * CH + tt * 128:c * CH + (tt + 1) * 128,
                                      d * CH:(d + 1) * CH],
                            in_=st,
                        )

            # Emission order: phase1(c+1) sits between attention(c)'s tail
            # and outproj(c), keeping the PE dense across phase boundaries.
            phase1(0)
            attention(0)
            for c in range(1, NCH):
                phase1(c)
                outproj(c - 1)
                attention(c)
            outproj(NCH - 1)
    return nc


def make_in_maps(x, W_qkv, b_qkv, W_out):
    """Host-side sharding: per-core input dict."""
    x = np.asarray(x, dtype=np.float32)
    W_qkv = np.asarray(W_qkv, dtype=np.float32)
    b_qkv = np.asarray(b_qkv, dtype=np.float32)
    W_out = np.asarray(W_out, dtype=np.float32)

    maskm = np.zeros((128, 896), np.float32)
    pp, cc = np.mgrid[0:128, 0:896]
    maskm[(cc - pp) >= 384] = 1.0

    in_maps = []
    for core in range(8):
        b, g = core // 4, core % 4
        heads = [4 * g + i for i in range(HPG)]
        xT = np.ascontiguousarray(x[b].T)  # [D, S]

        qk_cols, v_cols = [], []
        for h in heads:
            base = h * 3 * HD
            qk_cols.extend(range(base, base + HD))          # q
            qk_cols.extend(range(base + HD, base + 2 * HD))  # k
            v_cols.extend(range(base + 2 * HD, base + 3 * HD))
        qk_cols = np.array(qk_cols)
        v_cols = np.array(v_cols)

        wqk = np.ascontiguousarray(W_qkv[:, qk_cols])   # [D, 1024]
        wv = np.ascontiguousarray(W_qkv[:, v_cols])     # [D, 512]
        rows = np.concatenate([np.arange(h * HD, (h + 1) * HD) for h in heads])
        wout = np.ascontiguousarray(W_out[rows, :])     # [512, D]

        bqk = np.ascontiguousarray(
            b_qkv[qk_cols].reshape(2 * HPG, 128).T)     # [128, 8]
        bv = np.tile(b_qkv[v_cols][None, :], (128, 1))  # [128, 512]

        bf = ml_dtypes.bfloat16
        in_maps.append({
            "xT": xT.astype(bf), "wqk": wqk.astype(bf), "wv": wv.astype(bf),
            "wout": wout.astype(bf),
            "bqk": bqk, "bv": bv, "maskm": maskm.astype(bf),
            "onesf": np.ones((128, 128), ml_dtypes.bfloat16),
        })
    return in_maps


def reduce_outputs(results, b_out):
    """Sum the 4 per-head-group partials per batch; add b_out."""
    b_out = np.asarray(b_out, dtype=np.float32)
    out = np.zeros((B, S, DIM), np.float32)
    for core in range(8):
        out[core // 4] += results[core]["out"]
    out += b_out[None, None, :]
    return out


def host_simulate(x, W_qkv, b_qkv, W_out, b_out):
    """Numpy mirror of the exact device decomposition (for layout checks)."""
    in_maps = make_in_maps(x, W_qkv, b_qkv, W_out)
    results = []
    for core in range(8):
        m = in_maps[core]
        xT, wqk, wv, wout = m["xT"], m["wqk"], m["wv"], m["wout"]
        bqk, bv, maskm = m["bqk"], m["bv"], m["maskm"]
        qkT = wqk.T @ xT + bqk.T.reshape(-1, 1)  # [1024, S]
        v = xT.T @ wv + bv[0][None, :]           # [S, 512]
        out = np.zeros((S, DIM), np.float32)
        attnT = np.zeros((512, S), np.float32)
        for h in range(HPG):
            qT = qkT[2 * h * 128:(2 * h + 1) * 128]      # [128, S]
            kT = qkT[(2 * h + 1) * 128:(2 * h + 2) * 128]
            for c in range(NCH):
                nsk = 4 * (c + 1)
                q_c = qT[:, c * CH:(c + 1) * CH]
                ps_av = np.zeros((128, CH), np.float32)
                ps_d = np.zeros((1, CH), np.float32)
                for j in range(nsk):
                    sT = kT[:, j * 128:(j + 1) * 128].T @ q_c
                    pT = np.exp(sT * SCALE)
                    jj = j - 4 * c
                    if jj >= 0:
                        off = 384 - 128 * jj
                        pT = pT * maskm[:, off:off + CH]
                    ps_d += pT.sum(axis=0, keepdims=True)
                    ps_av += v[j * 128:(j + 1) * 128,
                               h * 128:(h + 1) * 128].T @ pT
                attnT[h * 128:(h + 1) * 128, c * CH:(c + 1) * CH] = (
                    ps_av / ps_d)
        out = attnT.T @ wout  # [S, D]
        results.append({"out": out})
    return reduce_outputs(results, b_out)


_NC = None


def kernel(x, mask, W_qkv, b_qkv, W_out, b_out):
    global _NC
    from concourse.bass_utils import run_bass_kernel_spmd
    if _NC is None:
        _NC = build_nc()
    in_maps = make_in_maps(x, W_qkv, b_qkv, W_out)
    res = run_bass_kernel_spmd(_NC, in_maps, core_ids=list(range(8)))
    return reduce_outputs(res.results, b_out)
